# revision 3
# baseline (speedup 1.0000x reference)
"""GCN 2-layer encoder on 8 Trainium2 NeuronCores (Bass/Tile), v2.

Push-sharded: core c owns src slice [25000c, 25000(c+1)).  Per layer, each
core builds a feature-major table T^T = (h*dinv) @ W in SBUF (f16 pairs
packed as f32 for L1, f32 for L2), then aggregates messages for ALL dst
nodes with GPSIMD ap_gather (SBUF->SBUF, 8 idx streams, d=1 f32) over a
host-built ELL schedule, tree-adds (DVE) reduce each dst's K slots, partial
sums accumulate per-owner in SBUF, a per-stream ap_gather unpermutes from
per-core degree-sorted rank order to a shared order, and ReduceScatter
(one per src-shard pass, overlapped) sums partials across cores.
Final output is assembled (unpermuted/transposed) on host.
"""
import numpy as np

import concourse.bass as bass
import concourse.bacc as bacc
import concourse.mybir as mybir
import concourse.tile as tile
from concourse.bass_utils import run_bass_kernel_spmd

N = 200000
E = 6400000
F_IN, F_HID, F_OUT = 32, 32, 16
NC = 8
SL = N // NC            # 25000 real nodes per owner
LP = 25088              # padded slice (196*128)
NSTR = 8                # idx streams (16-partition groups)
SPC = LP // NSTR        # 3136 shared cols per (owner, stream)
GRP = 64                # ELL rank-group granularity
NGO = SPC // GRP        # 49 rank groups per stream
RSH = SL // 2           # 12500: L1 shard split on raw local ids
SHARD = LP // 2         # 12544: table columns per pass
NI_CAP1 = 11776         # max gather idxs per call, layer 1 (f16-pair msgs)
NI_CAP2 = 11520         # layer 2 (f32 msgs)

f32, f16, i16 = mybir.dt.float32, mybir.dt.float16, mybir.dt.int16
AF = mybir.ActivationFunctionType


# ----------------------------------------------------------------- host prep
def _make_calls(K, cap):
    """K: [2, NC, NGO] -> list of calls (pass-major, owner asc, group asc).
    call = dict(h, ni, off, segs=[(owner, g0, nn, K, acc_off, first)],
    done=[o...]). Segments merge adjacent equal-K full groups of one owner."""
    calls = []
    for h in (0, 1):
        # flat list of (owner, acc_off, nnodes, K) units; split nodes freely
        units = []
        for o in range(NC):
            for g in range(NGO):
                kk = int(K[h, o, g])
                if kk == 0:
                    continue
                if units and units[-1][0] == o and units[-1][3] == kk \
                        and units[-1][1] + units[-1][2] == GRP * g:
                    o0, a0, nn, k0 = units[-1]
                    units[-1] = (o0, a0, nn + GRP, k0)
                else:
                    units.append((o, GRP * g, GRP, kk))
        packs, cur, w = [], [], 0
        ui = 0
        cap_use = cap - 15
        units = [list(u) for u in units]
        while ui < len(units):
            o, a0, nn, kk = units[ui]
            room = (cap_use - w) // kk
            if room < 1:
                packs.append(cur)
                cur, w = [], 0
                continue
            take = min(nn, room)
            cur.append((o, a0, take, kk))
            w += take * kk
            if take == nn:
                ui += 1
            else:
                units[ui] = [o, a0 + take, nn - take, kk]
        if cur:
            packs.append(cur)
        last_pack_of_owner = {}
        for pi, pk in enumerate(packs):
            for (o, a0, nn, kk) in pk:
                last_pack_of_owner[o] = pi
        for pi, pk in enumerate(packs):
            segs = [(o, a0 // GRP, nn, kk, a0, True) for (o, a0, nn, kk) in pk]
            done = [o for o, pi2 in last_pack_of_owner.items() if pi2 == pi]
            ni = sum(s[2] * s[3] for s in segs)
            nip = (ni + 15) // 16 * 16
            calls.append(dict(h=h, ni=ni, nip=nip, segs=segs,
                              done=sorted(done)))
    off = 0
    for cl in calls:
        cl["off"] = off
        off += cl["nip"] // 16
    return calls


def prepare(x, edge_index, W1, b1, W2, b2):
    x = np.asarray(x, np.float32)
    src = np.asarray(edge_index[0], dtype=np.int64)
    dst = np.asarray(edge_index[1], dtype=np.int64)
    W1 = np.asarray(W1, np.float32)
    W2 = np.asarray(W2, np.float32)
    b1 = np.asarray(b1, np.float32)
    b2 = np.asarray(b2, np.float32)

    deg = (np.bincount(dst, minlength=N) + 1).astype(np.float64)
    dinv = (1.0 / np.sqrt(deg)).astype(np.float32)

    # shared stream/col assignment per owner (snake-deal by total deg desc)
    scol_of_local = np.empty((NC, LP), np.int64)
    for o in range(NC):
        cnt = np.zeros(LP, np.int64)
        cnt[:SL] = deg[o * SL:(o + 1) * SL]
        order = np.argsort(-cnt, kind="stable")
        sc = (np.arange(LP) % NSTR) * SPC + (np.arange(LP) // NSTR)
        scol_of_local[o, order] = sc
    PAD1 = SHARD - 1                            # xt col 12543 is zero
    PAD2 = SHARD                                # explicit zero col in tab2

    K1 = np.zeros((2, NC, NGO), np.int64)
    K2 = np.zeros((2, NC, NGO), np.int64)
    so = src // SL
    eorder = np.argsort(so, kind="stable")
    bounds = np.searchsorted(so[eorder], np.arange(NC + 1))

    cores = []
    for c in range(NC):
        e = eorder[bounds[c]:bounds[c + 1]]
        s_loc = np.concatenate([src[e] - SL * c, np.arange(SL, dtype=np.int64)])
        d_glob = np.concatenate([dst[e], np.arange(SL, dtype=np.int64) + SL * c])
        h1p = s_loc // RSH
        col1 = s_loc - RSH * h1p
        sc_src = scol_of_local[c, s_loc]
        c2 = (sc_src % SPC) * NSTR + sc_src // SPC   # m-major column id
        h2p = c2 // SHARD
        col2 = c2 - SHARD * h2p
        do = d_glob // SL
        dl = d_glob - SL * do
        dsc = scol_of_local[do, dl]
        seg = do * NSTR + dsc // SPC            # (owner, stream) 0..63
        dm = dsc - (dsc // SPC) * SPC
        gkey = seg * SPC + dm                   # 0..200703

        # per (layer, pass) private rank order: sort by per-pass count desc,
        # then within each GRP-rank block reorder by min gathered column so
        # gather idx streams are locally ascending (SBUF-read locality).
        drs = {}
        rank_ofs = {}
        for (L, K, hp, colv) in ((1, K1, h1p, col1), (2, K2, h2p, col2)):
            dr = np.empty(len(s_loc), np.int64)
            for h in (0, 1):
                sel = hp == h
                chp = np.bincount(gkey[sel],
                                  minlength=NC * LP).reshape(NC * NSTR, SPC)
                rk = np.argsort(-chp, axis=1, kind="stable")
                cr = np.take_along_axis(chp, rk, axis=1)
                gmax = cr.reshape(NC, NSTR, NGO, GRP).max(axis=(1, 3))
                np.maximum(K[h], gmax, out=K[h])
                # min column per (seg, dm)
                key0 = gkey[sel]
                cs = colv[sel]
                srt = np.argsort(key0 * (1 << 16) + cs, kind="stable")
                ks, vs = key0[srt], cs[srt]
                first = np.r_[True, ks[1:] != ks[:-1]]
                minc = np.full(NC * LP, 1 << 20, np.int64)
                minc[ks[first]] = vs[first]
                minc = minc.reshape(NC * NSTR, SPC)
                vals = np.take_along_axis(minc, rk, axis=1)
                ord_in = np.argsort(
                    vals.reshape(NC * NSTR, NGO, GRP), axis=2, kind="stable")
                rk = np.take_along_axis(
                    rk.reshape(NC * NSTR, NGO, GRP), ord_in,
                    axis=2).reshape(NC * NSTR, SPC)
                rank_of = np.empty_like(rk)
                np.put_along_axis(rank_of, rk, np.broadcast_to(
                    np.arange(SPC)[None, :], rk.shape).copy(), axis=1)
                dr[sel] = rank_of[seg[sel], dm[sel]]
                rank_ofs[(L, h)] = rank_of
            drs[L] = dr
        cores.append(dict(h1p=h1p, col1=col1, h2p=h2p, col2=col2,
                          seg=seg, dr1=drs[1], dr2=drs[2],
                          rank_ofs=rank_ofs))

    calls1 = _make_calls(K1, NI_CAP1)
    calls2 = _make_calls(K2, NI_CAP2)
    C1 = sum(cl["nip"] for cl in calls1) // 16
    C2 = sum(cl["nip"] for cl in calls2) // 16
    zero_rngs = {1: {}, 2: {}}
    for L, K in ((1, K1), (2, K2)):
        for h in (0, 1):
            for o in range(NC):
                rng = []
                for g in range(NGO):
                    if K[h, o, g] == 0:
                        if rng and rng[-1][0] + rng[-1][1] == GRP * g:
                            rng[-1] = (rng[-1][0], rng[-1][1] + GRP)
                        else:
                            rng.append((GRP * g, GRP))
                zero_rngs[L][(h, o)] = rng
    schedule = dict(K1=K1, K2=K2, calls1=calls1, calls2=calls2,
                    C1=C1, C2=C2, zero_rngs=zero_rngs,
                    b1z=bool(not np.any(b1)))

    # ---------------- per-core runtime data
    in_maps = []
    for c in range(NC):
        cc = cores[c]

        def build_gidx(hp, colv, calls, padcols, dr):
            key = (hp * (NC * LP) + cc["seg"] * SPC + dr).astype(np.int64)
            o2 = np.lexsort((colv, key))
            cols_sorted = colv[o2]
            cnt = np.bincount(key, minlength=2 * NC * LP)
            ptr = np.zeros(2 * NC * LP + 1, np.int64)
            np.cumsum(cnt, out=ptr[1:])
            tiles = []
            for cl in calls:
                h, ni, nip = cl["h"], cl["ni"], cl["nip"]
                til = np.empty((128, nip // 16), np.int16)
                for s in range(NSTR):
                    parts = []
                    for (o, g0, nn, K, a0, first) in cl["segs"]:
                        base = h * (NC * LP) + (o * NSTR + s) * SPC
                        pos = base + a0 + np.arange(nn)
                        take = ptr[pos][:, None] + np.arange(K)[None, :]
                        valid = np.arange(K)[None, :] < cnt[pos][:, None]
                        vals = np.where(
                            valid,
                            cols_sorted[np.minimum(take, len(cols_sorted) - 1)],
                            padcols[h])
                        parts.append(vals.ravel())
                    parts.append(np.full(nip - ni, padcols[h], np.int64))
                    sv = np.concatenate(parts)
                    til[16 * s:16 * s + 16] = sv.reshape(nip // 16, 16).T
                tiles.append(til)
            return np.concatenate(tiles, axis=1)

        gidx1 = build_gidx(cc["h1p"], cc["col1"], calls1, (PAD1, PAD1),
                           cc["dr1"])
        gidx2 = build_gidx(cc["h2p"], cc["col2"], calls2, (PAD2, PAD2),
                           cc["dr2"])

        # uidx blocks ordered (L1A, L1B, L2A, L2B), each [128, NC*196]
        uidx = np.empty((128, 4 * NC * (SPC // 16)), np.int16)
        for bi, (L, h) in enumerate(((1, 0), (1, 1), (2, 0), (2, 1))):
            ro = cc["rank_ofs"][(L, h)]
            for o in range(NC):
                for s in range(NSTR):
                    r = ro[o * NSTR + s]             # [m] -> rank
                    uidx[16 * s:16 * s + 16,
                         (bi * NC + o) * (SPC // 16):
                         (bi * NC + o + 1) * (SPC // 16)] = \
                        r.reshape(SPC // 16, 16).T
        xt = np.zeros((F_IN, 2 * SHARD), np.float16)
        xs = (x[c * SL:(c + 1) * SL] * dinv[c * SL:(c + 1) * SL, None]).T
        xt[:, :RSH] = xs[:, :RSH]
        xt[:, SHARD:SHARD + RSH] = xs[:, RSH:]
        d_ord = np.ones(LP, np.float32)
        loc = np.argsort(scol_of_local[c])          # scol -> local id
        real = loc < SL
        d_ord[real] = dinv[c * SL + loc[real]]
        dinv2 = np.empty((128, SPC), np.float32)
        for s in range(NSTR):
            dinv2[16 * s:16 * s + 16] = d_ord[s * SPC:(s + 1) * SPC][None, :]
        b1z = not np.any(b1)
        # pre-table scale tile in pair layout: dinv^2 if b1==0 else dinv
        dvt = (dinv2 * dinv2 if b1z else dinv2).astype(np.float16)
        fp = np.arange(128) % 16
        w1a = W1[:, 2 * fp].astype(np.float16)
        w1b = W1[:, 2 * fp + 1].astype(np.float16)
        # w2v[q]: q=2*parity+j, nonzero rows [16*parity,16*parity+16):
        #   row 16*parity+fp = W2[2fp+j, col%16]
        w2v = np.zeros((4, 32, 128), np.float16)
        for par in (0, 1):
            for j in (0, 1):
                q = 2 * par + j
                for fpp in range(16):
                    w2v[q, 16 * par + fpp] = W2[2 * fpp + j][
                        np.arange(128) % 16]
        b2P = np.tile(b2, 8).reshape(128, 1).astype(np.float32)
        im = {
            "xt": xt, "gidx1": gidx1, "gidx2": gidx2, "uidx": uidx,
            "dvt": dvt, "dinv2": dinv2,
            "w1a": w1a, "w1b": w1b, "w2v": w2v, "b2P": b2P,
        }
        if not b1z:
            b1big = np.empty((128, SPC, 2), np.float16)
            for j in (0, 1):
                b1big[:, :, j] = b1[2 * (np.arange(128)[:, None] % 16) + j]
            im["b1big"] = b1big
        in_maps.append(im)
    meta = dict(scol_of_local=scol_of_local)
    return in_maps, schedule, meta


# ----------------------------------------------------------------- build
def _tree_reduce(nc, v, K, final_out):
    """Sum the k axis of v [p, nn, K, j]; the last add writes final_out."""
    ops = []
    k = K
    while k > 1:
        if k % 2 == 1:
            ops.append((0, 1, k - 1, k))
            k -= 1
        half = k // 2
        ops.append((0, half, half, k))
        k = half
    for i, (o0, o1, i0, i1) in enumerate(ops):
        a = v[:, :, o0:o1, :]
        b = v[:, :, i0:i1, :]
        if i == len(ops) - 1:
            nc.vector.tensor_add(out=final_out, in0=a, in1=b)
        else:
            nc.vector.tensor_add(out=a, in0=a, in1=b)


def build(schedule):
    calls1, calls2 = schedule["calls1"], schedule["calls2"]
    C1, C2 = schedule["C1"], schedule["C2"]
    zero_rngs = schedule["zero_rngs"]

    nc = bacc.Bacc("TRN2", target_bir_lowering=False, debug=False,
                   num_devices=NC)
    xt = nc.dram_tensor("xt", [F_IN, 2 * SHARD], f16, kind="ExternalInput").ap()
    gidx1 = nc.dram_tensor("gidx1", [128, C1], i16, kind="ExternalInput").ap()
    gidx2 = nc.dram_tensor("gidx2", [128, C2], i16, kind="ExternalInput").ap()
    uidx = nc.dram_tensor("uidx", [128, 4 * NC * (SPC // 16)], i16,
                          kind="ExternalInput").ap()
    dvt = nc.dram_tensor("dvt", [128, SPC], f16, kind="ExternalInput").ap()
    dinv2 = nc.dram_tensor("dinv2", [128, SPC], f32, kind="ExternalInput").ap()
    w1a = nc.dram_tensor("w1a", [F_IN, 128], f16, kind="ExternalInput").ap()
    w1b = nc.dram_tensor("w1b", [F_IN, 128], f16, kind="ExternalInput").ap()
    w2v = nc.dram_tensor("w2v", [4, 32, 128], f16, kind="ExternalInput").ap()
    b2P = nc.dram_tensor("b2P", [128, 1], f32, kind="ExternalInput").ap()
    b1z = schedule["b1z"]
    b1big = (None if b1z else nc.dram_tensor(
        "b1big", [128, SPC, 2], f16, kind="ExternalInput").ap())
    out = nc.dram_tensor("out", [128, SPC], f32, kind="ExternalOutput").ap()

    part1 = [nc.dram_tensor(f"part1{h}", [NC, 128, SPC, 2], f16).ap()
             for h in (0, 1)]
    rs1 = [nc.dram_tensor(f"rs1{h}", [128, SPC, 2], f16).ap() for h in (0, 1)]
    part2 = [nc.dram_tensor(f"part2{h}", [NC, 128, SPC], f32).ap()
             for h in (0, 1)]
    rs2 = [nc.dram_tensor(f"rs2{h}", [128, SPC], f32).ap() for h in (0, 1)]

    with tile.TileContext(nc) as tc:
        with tc.tile_pool(name="const", bufs=1) as const, \
             tc.tile_pool(name="psum", bufs=4, space="PSUM") as psp:
            w1at = const.tile([F_IN, 128], f16)
            nc.sync.dma_start(out=w1at[:], in_=w1a[:])
            w1bt = const.tile([F_IN, 128], f16)
            nc.sync.dma_start(out=w1bt[:], in_=w1b[:])
            w2vt = []
            for q in range(4):
                wv = const.tile([32, 128], f16, name=f"w2v{q}", tag=f"w2v{q}")
                nc.sync.dma_start(out=wv[:], in_=w2v[q])
                w2vt.append(wv)
            b2t = const.tile([128, 1], f32)
            nc.sync.dma_start(out=b2t[:], in_=b2P[:])
            uix = const.tile([128, 4 * NC * (SPC // 16)], i16)
            nc.sync.dma_start(out=uix[:], in_=uidx[:])

            def run_layer(L, calls, gidx, part, rsl, cap, tab_builder):
                acc_t = {}
                def emit_cc(h):
                    bass.BassGpSimd.collective_compute(
                        nc.gpsimd, "ReduceScatter", mybir.AluOpType.add,
                        replica_groups=[list(range(NC))],
                        ins=[part[h][:]], outs=[rsl[h][:]])

                with tc.tile_pool(name=f"w{L}", bufs=1) as wp, \
                     tc.tile_pool(name=f"m{L}", bufs=2) as mp:
                    for h in (0, 1):
                        tab = wp.tile([128, SHARD + 8, 2], f16, tag="tab",
                                      bufs=1)
                        tab_builder(h, tab, mp)
                        if h == 1:
                            emit_cc(0)
                        pending = []
                        hcalls = [c for c in calls if c["h"] == h]
                        for cl in hcalls:
                            ni, nip = cl["ni"], cl["nip"]
                            ixt = mp.tile([128, cap // 16], i16, tag="gix",
                                          bufs=2)
                            nc.sync.dma_start(
                                out=ixt[:, :nip // 16],
                                in_=gidx[:, cl["off"]:cl["off"] + nip // 16])
                            msg = mp.tile([128, cap, 2], f16, tag="msg",
                                          bufs=2)
                            nc.gpsimd.ap_gather(
                                msg[:, :nip, :].bitcast(f32),
                                tab[:].bitcast(f32)[:, :SHARD + 8 * (L == 2)],
                                ixt[:, :nip // 16],
                                channels=128,
                                num_elems=SHARD + 8 * (L == 2), d=1,
                                num_idxs=nip)
                            off = 0
                            for (o, g0, nn, K, a0, first) in cl["segs"]:
                                if o not in acc_t:
                                    acc_t[o] = mp.tile([128, SPC, 2], f16, name=f"acc{o}",
                                                       tag="acc", bufs=2)
                                acc = acc_t[o]
                                if L == 1:
                                    v = msg[:, off:off + nn * K, :].rearrange(
                                        "p (n k) j -> p n k j", k=K)
                                    dstv = acc[:, a0:a0 + nn, :]
                                else:
                                    v = msg[:, off:off + nn * K, :].bitcast(
                                        f32).rearrange(
                                        "p (n k) u -> p n k u", k=K)
                                    dstv = acc[:].bitcast(f32)[:, a0:a0 + nn]
                                if K == 1:
                                    nc.vector.tensor_copy(dstv, v[:, :, 0, :])
                                else:
                                    _tree_reduce(nc, v, K, dstv)
                                off += nn * K
                            todo = pending
                            pending = cl["done"]
                            if cl is hcalls[-1]:
                                todo = todo + pending
                                pending = []
                            for o in todo:
                                acc = acc_t.pop(o)
                                for (z0, zn) in zero_rngs[L][(h, o)]:
                                    nc.vector.memset(acc[:, z0:z0 + zn, :], 0)
                                S = mp.tile([128, SPC, 2], f16, tag="S",
                                            bufs=1)
                                bi = (L - 1) * 2 + h
                                nc.gpsimd.ap_gather(
                                    S[:].bitcast(f32), acc[:].bitcast(f32),
                                    uix[:, (bi * NC + o) * (SPC // 16):
                                        (bi * NC + o + 1) * (SPC // 16)],
                                    channels=128, num_elems=SPC, d=1,
                                    num_idxs=SPC)
                                if L == 1:
                                    nc.sync.dma_start(
                                        out=part[h][o].rearrange(
                                            "p m j -> p (m j)"),
                                        in_=S[:].rearrange(
                                            "p m j -> p (m j)"))
                                else:
                                    nc.sync.dma_start(
                                        out=part[h][o],
                                        in_=S[:].bitcast(f32))
                    emit_cc(1)

            def tab1_builder(h, tab, mp):
                for q0 in range(0, SHARD, SPC):
                    xs = mp.tile([F_IN, SPC], f16, tag="xsrc", bufs=2)
                    nc.sync.dma_start(out=xs[:],
                                      in_=xt[:, h * SHARD + q0:
                                             h * SHARD + q0 + SPC])
                    for c0 in range(0, SPC, 512):
                        cw = min(512, SPC - c0)
                        for j, wt in ((0, w1at), (1, w1bt)):
                            ps = psp.tile([128, 512], f32, tag="ps")
                            nc.tensor.matmul(ps[:, :cw], lhsT=wt[:],
                                             rhs=xs[:, c0:c0 + cw],
                                             start=True, stop=True)
                            if j == 0:
                                nc.scalar.activation(
                                    tab[:, q0 + c0:q0 + c0 + cw, j],
                                    ps[:, :cw], AF.Copy)
                            else:
                                nc.vector.tensor_copy(
                                    tab[:, q0 + c0:q0 + c0 + cw, j],
                                    ps[:, :cw])

            def tab2_builder(h, tab, mp):
                CW = 512
                HM = SPC // 2                    # 1568 m-positions per shard
                tabv = tab[:].bitcast(f32).rearrange("p (m s) u -> p m (s u)", s=8)
                nc.vector.memset(tabv[:, HM, :], 0)   # zero pad cols
                for b in range(4):
                    p0 = 32 * b
                    dvh = mp.tile([32, SPC // 2], f16, tag="dsq", bufs=2)
                    nc.sync.dma_start(
                        out=dvh[:],
                        in_=dvt[p0:p0 + 32,
                                (SPC // 2) * h:(SPC // 2) * (h + 1)])
                    for c0 in range(0, HM, CW):
                        cw = min(CW, HM - c0)
                        cm = HM * h + c0
                        t0 = mp.tile([32, CW, 2], f16, tag="h1c", bufs=2)
                        nc.sync.dma_start(
                            out=t0[:, :cw, :],
                            in_=rs1[0][p0:p0 + 32, cm:cm + cw, :])
                        t1 = mp.tile([32, CW, 2], f16, tag="h1d", bufs=2)
                        nc.sync.dma_start(
                            out=t1[:, :cw, :],
                            in_=rs1[1][p0:p0 + 32, cm:cm + cw, :])
                        nc.vector.tensor_add(out=t0[:, :cw, :],
                                             in0=t0[:, :cw, :],
                                             in1=t1[:, :cw, :])
                        dvb = dvh[:, c0:c0 + cw].broadcast_to([32, cw, 2])
                        nc.vector.tensor_mul(out=t0[:, :cw, :],
                                             in0=t0[:, :cw, :], in1=dvb)
                        if not b1z:
                            bb = mp.tile([32, CW, 2], f16, tag="b1c", bufs=2)
                            nc.sync.dma_start(
                                out=bb[:, :cw, :],
                                in_=b1big[p0:p0 + 32, cm:cm + cw, :])
                            nc.vector.tensor_add(out=t0[:, :cw, :],
                                                 in0=t0[:, :cw, :],
                                                 in1=bb[:, :cw, :])
                        nc.scalar.activation(t0[:, :cw, :], t0[:, :cw, :],
                                             AF.Relu)
                        if not b1z:
                            nc.vector.tensor_mul(out=t0[:, :cw, :],
                                                 in0=t0[:, :cw, :], in1=dvb)
                        for par in (0, 1):
                            u = 2 * b + par
                            ps = psp.tile([128, CW], f32, tag="ps")
                            nc.tensor.matmul(
                                ps[:, :cw], lhsT=w2vt[2 * par][:],
                                rhs=t0[:, :cw, 0],
                                start=True, stop=False)
                            nc.tensor.matmul(
                                ps[:, :cw], lhsT=w2vt[2 * par + 1][:],
                                rhs=t0[:, :cw, 1],
                                start=False, stop=True)
                            nc.scalar.activation(
                                tabv[:, c0:c0 + cw, u], ps[:, :cw], AF.Copy)

            run_layer(1, calls1, gidx1, part1, rs1, NI_CAP1, tab1_builder)
            run_layer(2, calls2, gidx2, part2, rs2, NI_CAP2, tab2_builder)

            with tc.tile_pool(name="fin", bufs=1) as fpool:
                dv2 = fpool.tile([128, SPC], f32, tag="dv2")
                nc.sync.dma_start(out=dv2[:], in_=dinv2[:])
                o0 = fpool.tile([128, SPC, 2], f16, tag="fo")
                nc.sync.dma_start(out=o0[:].bitcast(f32), in_=rs2[0][:])
                nc.vector.tensor_mul(out=o0[:].bitcast(f32),
                                     in0=o0[:].bitcast(f32), in1=dv2[:])
                nc.vector.tensor_scalar_add(o0[:].bitcast(f32),
                                            o0[:].bitcast(f32), b2t[:])
                o1 = fpool.tile([128, SPC, 2], f16, tag="fo2")
                HF = SPC // 2
                for z in (0, 1):
                    sl = slice(z * HF, (z + 1) * HF)
                    nc.sync.dma_start(out=o1[:].bitcast(f32)[:, sl],
                                      in_=rs2[1][:, sl])
                    nc.vector.tensor_mul(out=o1[:].bitcast(f32)[:, sl],
                                         in0=o1[:].bitcast(f32)[:, sl],
                                         in1=dv2[:, sl])
                    nc.vector.tensor_add(out=o0[:].bitcast(f32)[:, sl],
                                         in0=o0[:].bitcast(f32)[:, sl],
                                         in1=o1[:].bitcast(f32)[:, sl])
                    nc.sync.dma_start(out=out[:, sl],
                                      in_=o0[:].bitcast(f32)[:, sl])
    nc.compile()
    return nc


# ----------------------------------------------------------------- wrapper
_CACHE = {}


def kernel(x, edge_index, W1, b1, W2, b2):
    in_maps, schedule, meta = prepare(x, edge_index, W1, b1, W2, b2)
    key = (schedule["K1"].tobytes() + schedule["K2"].tobytes() + bytes([schedule["b1z"]]))
    if key not in _CACHE:
        _CACHE[key] = build(schedule)
    nc = _CACHE[key]
    res = run_bass_kernel_spmd(nc, in_maps, list(range(NC)))
    scol = meta["scol_of_local"]
    full = np.empty((N, F_OUT), np.float32)
    for c in range(NC):
        outc = np.asarray(res.results[c]["out"])     # [128, SPC]
        sc = scol[c, :SL]
        s, m = sc // SPC, sc % SPC
        full[c * SL:(c + 1) * SL] = outc[
            (16 * s[:, None] + np.arange(F_OUT)[None, :]), m[:, None]]
    return full



# revision 14
# speedup vs baseline: 1.0809x; 1.0809x over previous
"""GCN 2-layer encoder on 8 Trainium2 NeuronCores (Bass/Tile), v2.

Push-sharded: core c owns src slice [25000c, 25000(c+1)).  Per layer, each
core builds a feature-major table T^T = (h*dinv) @ W in SBUF (f16 pairs
packed as f32 for L1, f32 for L2), then aggregates messages for ALL dst
nodes with GPSIMD ap_gather (SBUF->SBUF, 8 idx streams, d=1 f32) over a
host-built ELL schedule, tree-adds (DVE) reduce each dst's K slots, partial
sums accumulate per-owner in SBUF, a per-stream ap_gather unpermutes from
per-core degree-sorted rank order to a shared order, and ReduceScatter
(one per src-shard pass, overlapped) sums partials across cores.
Final output is assembled (unpermuted/transposed) on host.
"""
import numpy as np

import concourse.bass as bass
import concourse.bacc as bacc
import concourse.mybir as mybir
import concourse.tile as tile
from concourse.bass_utils import run_bass_kernel_spmd

N = 200000
E = 6400000
F_IN, F_HID, F_OUT = 32, 32, 16
NC = 8
SL = N // NC            # 25000 real nodes per owner
LP = 25088              # padded slice (196*128)
NSTR = 8                # idx streams (16-partition groups)
SPC = LP // NSTR        # 3136 shared cols per (owner, stream)
GRP = 64                # ELL rank-group granularity
NGO = SPC // GRP        # 49 rank groups per stream
RSH = SL // 2           # 12500: L1 shard split on raw local ids
SHARD = LP // 2         # 12544: table columns per pass
NI_CAP1 = 11776         # max gather idxs per call, layer 1 (f16-pair msgs)
NI_CAP2 = 11520         # layer 2 (f32 msgs)

f32, f16, i16 = mybir.dt.float32, mybir.dt.float16, mybir.dt.int16
AF = mybir.ActivationFunctionType
LOCALITY = True          # sort gather idx for SBUF-read locality


# ----------------------------------------------------------------- host prep
def _make_calls(K, cap):
    """K: [2, NC, NGO] -> list of calls (pass-major, owner asc, group asc).
    call = dict(h, ni, off, segs=[(owner, g0, nn, K, acc_off, first)],
    done=[o...]). Segments merge adjacent equal-K full groups of one owner."""
    calls = []
    for h in (0, 1):
        # flat list of (owner, acc_off, nnodes, K) units; split nodes freely
        units = []
        for o in range(NC):
            for g in range(NGO):
                kk = int(K[h, o, g])
                if kk == 0:
                    continue
                if units and units[-1][0] == o and units[-1][3] == kk \
                        and units[-1][1] + units[-1][2] == GRP * g:
                    o0, a0, nn, k0 = units[-1]
                    units[-1] = (o0, a0, nn + GRP, k0)
                else:
                    units.append((o, GRP * g, GRP, kk))
        packs, cur, w = [], [], 0
        ui = 0
        cap_use = cap - 15
        units = [list(u) for u in units]
        while ui < len(units):
            o, a0, nn, kk = units[ui]
            room = (cap_use - w) // kk
            if room < 1:
                packs.append(cur)
                cur, w = [], 0
                continue
            take = min(nn, room)
            cur.append((o, a0, take, kk))
            w += take * kk
            if take == nn:
                ui += 1
            else:
                units[ui] = [o, a0 + take, nn - take, kk]
        if cur:
            packs.append(cur)
        last_pack_of_owner = {}
        for pi, pk in enumerate(packs):
            for (o, a0, nn, kk) in pk:
                last_pack_of_owner[o] = pi
        for pi, pk in enumerate(packs):
            segs = [(o, a0 // GRP, nn, kk, a0, True) for (o, a0, nn, kk) in pk]
            done = [o for o, pi2 in last_pack_of_owner.items() if pi2 == pi]
            ni = sum(s[2] * s[3] for s in segs)
            nip = (ni + 15) // 16 * 16
            calls.append(dict(h=h, ni=ni, nip=nip, segs=segs,
                              done=sorted(done)))
    off = 0
    for cl in calls:
        cl["off"] = off
        off += cl["nip"] // 16
    return calls


def prepare(x, edge_index, W1, b1, W2, b2):
    x = np.asarray(x, np.float32)
    src = np.asarray(edge_index[0], dtype=np.int64)
    dst = np.asarray(edge_index[1], dtype=np.int64)
    W1 = np.asarray(W1, np.float32)
    W2 = np.asarray(W2, np.float32)
    b1 = np.asarray(b1, np.float32)
    b2 = np.asarray(b2, np.float32)

    deg = (np.bincount(dst, minlength=N) + 1).astype(np.float64)
    dinv = (1.0 / np.sqrt(deg)).astype(np.float32)

    # shared stream/col assignment per owner (snake-deal by total deg desc)
    scol_of_local = np.empty((NC, LP), np.int64)
    for o in range(NC):
        cnt = np.zeros(LP, np.int64)
        cnt[:SL] = deg[o * SL:(o + 1) * SL]
        order = np.argsort(-cnt, kind="stable")
        sc = (np.arange(LP) % NSTR) * SPC + (np.arange(LP) // NSTR)
        scol_of_local[o, order] = sc
    PAD1 = SHARD - 1                            # xt col 12543 is zero
    PAD2 = SHARD                                # explicit zero col in tab2

    K1 = np.zeros((2, NC, NGO), np.int64)
    K2 = np.zeros((2, NC, NGO), np.int64)
    so = src // SL
    eorder = np.argsort(so, kind="stable")
    bounds = np.searchsorted(so[eorder], np.arange(NC + 1))

    cores = []
    for c in range(NC):
        e = eorder[bounds[c]:bounds[c + 1]]
        s_loc = np.concatenate([src[e] - SL * c, np.arange(SL, dtype=np.int64)])
        d_glob = np.concatenate([dst[e], np.arange(SL, dtype=np.int64) + SL * c])
        h1p = s_loc // RSH
        col1 = s_loc - RSH * h1p
        sc_src = scol_of_local[c, s_loc]
        c2 = (sc_src % SPC) * NSTR + sc_src // SPC   # m-major column id
        h2p = c2 // SHARD
        col2 = c2 - SHARD * h2p
        do = d_glob // SL
        dl = d_glob - SL * do
        dsc = scol_of_local[do, dl]
        seg = do * NSTR + dsc // SPC            # (owner, stream) 0..63
        dm = dsc - (dsc // SPC) * SPC
        gkey = seg * SPC + dm                   # 0..200703

        # per (layer, pass) private rank order: sort by per-pass count desc,
        # then within each GRP-rank block reorder by min gathered column so
        # gather idx streams are locally ascending (SBUF-read locality).
        drs = {}
        rank_ofs = {}
        for (L, K, hp, colv) in ((1, K1, h1p, col1), (2, K2, h2p, col2)):
            dr = np.empty(len(s_loc), np.int64)
            for h in (0, 1):
                sel = hp == h
                chp = np.bincount(gkey[sel],
                                  minlength=NC * LP).reshape(NC * NSTR, SPC)
                rk = np.argsort(-chp, axis=1, kind="stable")
                cr = np.take_along_axis(chp, rk, axis=1)
                gmax = cr.reshape(NC, NSTR, NGO, GRP).max(axis=(1, 3))
                np.maximum(K[h], gmax, out=K[h])
                if LOCALITY:
                    # min column per (seg, dm)
                    key0 = gkey[sel]
                    cs = colv[sel]
                    srt = np.argsort(key0 * (1 << 16) + cs, kind="stable")
                    ks, vs = key0[srt], cs[srt]
                    first = np.r_[True, ks[1:] != ks[:-1]]
                    minc = np.full(NC * LP, 1 << 20, np.int64)
                    minc[ks[first]] = vs[first]
                    minc = minc.reshape(NC * NSTR, SPC)
                    vals = np.take_along_axis(minc, rk, axis=1)
                    ord_in = np.argsort(
                        vals.reshape(NC * NSTR, NGO, GRP), axis=2,
                        kind="stable")
                    rk = np.take_along_axis(
                        rk.reshape(NC * NSTR, NGO, GRP), ord_in,
                        axis=2).reshape(NC * NSTR, SPC)
                rank_of = np.empty_like(rk)
                np.put_along_axis(rank_of, rk, np.broadcast_to(
                    np.arange(SPC)[None, :], rk.shape).copy(), axis=1)
                dr[sel] = rank_of[seg[sel], dm[sel]]
                rank_ofs[(L, h)] = rank_of
            drs[L] = dr
        cores.append(dict(h1p=h1p, col1=col1, h2p=h2p, col2=col2,
                          seg=seg, dr1=drs[1], dr2=drs[2],
                          rank_ofs=rank_ofs))

    calls1 = _make_calls(K1, NI_CAP1)
    calls2 = _make_calls(K2, NI_CAP2)
    C1 = sum(cl["nip"] for cl in calls1) // 16
    C2 = sum(cl["nip"] for cl in calls2) // 16
    zero_rngs = {1: {}, 2: {}}
    for L, K in ((1, K1), (2, K2)):
        for h in (0, 1):
            for o in range(NC):
                rng = []
                for g in range(NGO):
                    if K[h, o, g] == 0:
                        if rng and rng[-1][0] + rng[-1][1] == GRP * g:
                            rng[-1] = (rng[-1][0], rng[-1][1] + GRP)
                        else:
                            rng.append((GRP * g, GRP))
                zero_rngs[L][(h, o)] = rng
    schedule = dict(K1=K1, K2=K2, calls1=calls1, calls2=calls2,
                    C1=C1, C2=C2, zero_rngs=zero_rngs,
                    b1z=bool(not np.any(b1)))

    # ---------------- per-core runtime data
    in_maps = []
    for c in range(NC):
        cc = cores[c]

        def build_gidx(hp, colv, calls, padcols, dr):
            key = (hp * (NC * LP) + cc["seg"] * SPC + dr).astype(np.int64)
            o2 = np.lexsort((colv, key)) if LOCALITY \
                else np.argsort(key, kind="stable")
            cols_sorted = colv[o2]
            cnt = np.bincount(key, minlength=2 * NC * LP)
            ptr = np.zeros(2 * NC * LP + 1, np.int64)
            np.cumsum(cnt, out=ptr[1:])
            tiles = []
            for cl in calls:
                h, ni, nip = cl["h"], cl["ni"], cl["nip"]
                til = np.empty((128, nip // 16), np.int16)
                for s in range(NSTR):
                    parts = []
                    for (o, g0, nn, K, a0, first) in cl["segs"]:
                        base = h * (NC * LP) + (o * NSTR + s) * SPC
                        pos = base + a0 + np.arange(nn)
                        take = ptr[pos][:, None] + np.arange(K)[None, :]
                        valid = np.arange(K)[None, :] < cnt[pos][:, None]
                        vals = np.where(
                            valid,
                            cols_sorted[np.minimum(take, len(cols_sorted) - 1)],
                            padcols[h])
                        parts.append(vals.ravel())
                    parts.append(np.full(nip - ni, padcols[h], np.int64))
                    sv = np.concatenate(parts)
                    til[16 * s:16 * s + 16] = sv.reshape(nip // 16, 16).T
                tiles.append(til)
            return np.concatenate(tiles, axis=1)

        gidx1 = build_gidx(cc["h1p"], cc["col1"], calls1, (PAD1, PAD1),
                           cc["dr1"])
        gidx2 = build_gidx(cc["h2p"], cc["col2"], calls2, (PAD2, PAD2),
                           cc["dr2"])

        # uidx blocks ordered (L1A, L1B, L2A, L2B), each [128, NC*196]
        uidx = np.empty((128, 4 * NC * (SPC // 16)), np.int16)
        for bi, (L, h) in enumerate(((1, 0), (1, 1), (2, 0), (2, 1))):
            ro = cc["rank_ofs"][(L, h)]
            for o in range(NC):
                for s in range(NSTR):
                    r = ro[o * NSTR + s]             # [m] -> rank
                    uidx[16 * s:16 * s + 16,
                         (bi * NC + o) * (SPC // 16):
                         (bi * NC + o + 1) * (SPC // 16)] = \
                        r.reshape(SPC // 16, 16).T
        xt = np.zeros((F_IN, 2 * SHARD), np.float16)
        xs = (x[c * SL:(c + 1) * SL] * dinv[c * SL:(c + 1) * SL, None]).T
        xt[:, :RSH] = xs[:, :RSH]
        xt[:, SHARD:SHARD + RSH] = xs[:, RSH:]
        d_ord = np.ones(LP, np.float32)
        loc = np.argsort(scol_of_local[c])          # scol -> local id
        real = loc < SL
        d_ord[real] = dinv[c * SL + loc[real]]
        dinv2 = np.empty((128, SPC), np.float32)
        for s in range(NSTR):
            dinv2[16 * s:16 * s + 16] = d_ord[s * SPC:(s + 1) * SPC][None, :]
        b1z = not np.any(b1)
        # pre-table scale tile in pair layout: dinv^2 if b1==0 else dinv
        dvt = (dinv2 * dinv2 if b1z else dinv2).astype(np.float16)
        fp = np.arange(128) % 16
        w1a = W1[:, 2 * fp].astype(np.float16)
        w1b = W1[:, 2 * fp + 1].astype(np.float16)
        # w2v[q]: q=2*parity+j, nonzero rows [16*parity,16*parity+16):
        #   row 16*parity+fp = W2[2fp+j, col%16]
        w2v = np.zeros((4, 32, 128), np.float16)
        for par in (0, 1):
            for j in (0, 1):
                q = 2 * par + j
                for fpp in range(16):
                    w2v[q, 16 * par + fpp] = W2[2 * fpp + j][
                        np.arange(128) % 16]
        b2P = np.tile(b2, 8).reshape(128, 1).astype(np.float32)
        blobB = np.concatenate(
            [xt, w1a, w1b] + [w2v[q] for q in range(4)], axis=1)
        blobBp = np.zeros((128, blobB.shape[1]), np.float16)
        blobBp[:F_IN] = blobB
        blobA = np.concatenate(
            [np.ascontiguousarray(dinv2).view(np.int16),
             np.ascontiguousarray(b2P).view(np.int16),
             uidx, gidx1, gidx2,
             dvt.view(np.int16),
             blobBp.view(np.int16)], axis=1)
        if blobA.shape[1] % 2:
            blobA = np.concatenate(
                [blobA, np.zeros((128, 1), np.int16)], axis=1)
        im = {"blobA": blobA}
        if not b1z:
            b1big = np.empty((128, SPC, 2), np.float16)
            for j in (0, 1):
                b1big[:, :, j] = b1[2 * (np.arange(128)[:, None] % 16) + j]
            im["b1big"] = b1big
        in_maps.append(im)
    meta = dict(scol_of_local=scol_of_local)
    return in_maps, schedule, meta


# ----------------------------------------------------------------- build
def _tree_reduce(nc, v, K, final_out):
    """Sum the k axis of v [p, nn, K, j]; the last add writes final_out."""
    ops = []
    k = K
    while k > 1:
        if k % 2 == 1:
            ops.append((0, 1, k - 1, k))
            k -= 1
        half = k // 2
        ops.append((0, half, half, k))
        k = half
    for i, (o0, o1, i0, i1) in enumerate(ops):
        a = v[:, :, o0:o1, :]
        b = v[:, :, i0:i1, :]
        if i == len(ops) - 1:
            nc.vector.tensor_add(out=final_out, in0=a, in1=b)
        else:
            nc.vector.tensor_add(out=a, in0=a, in1=b)


def build(schedule):
    calls1, calls2 = schedule["calls1"], schedule["calls2"]
    C1, C2 = schedule["C1"], schedule["C2"]
    zero_rngs = schedule["zero_rngs"]

    nc = bacc.Bacc("TRN2", target_bir_lowering=False, debug=False,
                   num_devices=NC)
    UW = 4 * NC * (SPC // 16)
    WB = 2 * SHARD + 128 + 128 + 4 * 128
    WA = UW + C1 + C2 + SPC + 2 * SPC + 2 + WB
    WA += WA % 2
    blobA = nc.dram_tensor("blobA", [128, WA], i16, kind="ExternalInput").ap()
    off = 0
    dinv2 = blobA[:, off:off + 2 * SPC].bitcast(f32); off += 2 * SPC
    b2P = blobA[:, off:off + 2].bitcast(f32); off += 2
    uidx = blobA[:, off:off + UW]; off += UW
    gidx1 = blobA[:, off:off + C1]; off += C1
    gidx2 = blobA[:, off:off + C2]; off += C2
    dvt = blobA[:, off:off + SPC].bitcast(f16); off += SPC
    blobB = blobA[:F_IN, off:].bitcast(f16)
    off = 0
    xt = blobB[:, off:off + 2 * SHARD]; off += 2 * SHARD
    w1a = blobB[:, off:off + 128]; off += 128
    w1b = blobB[:, off:off + 128]; off += 128
    w2v = [blobB[:, off + q * 128:off + (q + 1) * 128] for q in range(4)]
    b1z = schedule["b1z"]
    b1big = (None if b1z else nc.dram_tensor(
        "b1big", [128, SPC, 2], f16, kind="ExternalInput").ap())
    out = nc.dram_tensor("out", [128, SPC], f32, kind="ExternalOutput").ap()

    part1 = [nc.dram_tensor(f"part1{h}", [NC, 128, SPC, 2], f16).ap()
             for h in (0, 1)]
    rs1 = [nc.dram_tensor(f"rs1{h}", [128, SPC, 2], f16).ap() for h in (0, 1)]
    part2 = [nc.dram_tensor(f"part2{h}", [NC, 128, SPC], f32).ap()
             for h in (0, 1)]
    rs2 = [nc.dram_tensor(f"rs2{h}", [128, SPC], f32).ap() for h in (0, 1)]

    with tile.TileContext(nc) as tc:
        with tc.tile_pool(name="const", bufs=1) as const, \
             tc.tile_pool(name="psum", bufs=4, space="PSUM") as psp:
            w1at = const.tile([F_IN, 128], f16)
            nc.sync.dma_start(out=w1at[:], in_=w1a[:])
            w1bt = const.tile([F_IN, 128], f16)
            nc.sync.dma_start(out=w1bt[:], in_=w1b[:])
            w2vt = []
            for q in range(4):
                wv = const.tile([32, 128], f16, name=f"w2v{q}", tag=f"w2v{q}")
                nc.sync.dma_start(out=wv[:], in_=w2v[q])
                w2vt.append(wv)
            b2t = const.tile([128, 1], f32)
            nc.sync.dma_start(out=b2t[:], in_=b2P[:])
            uix = const.tile([128, 4 * NC * (SPC // 16)], i16)
            nc.sync.dma_start(out=uix[:], in_=uidx[:])

            def run_layer(L, calls, gidx, part, rsl, cap, tab_builder):
                acc_t = {}
                def emit_cc(h):
                    bass.BassGpSimd.collective_compute(
                        nc.gpsimd, "ReduceScatter", mybir.AluOpType.add,
                        replica_groups=[list(range(NC))],
                        ins=[part[h][:]], outs=[rsl[h][:]])

                with tc.tile_pool(name=f"w{L}", bufs=1) as wp, \
                     tc.tile_pool(name=f"m{L}", bufs=2) as mp:
                    for h in (0, 1):
                        tab = wp.tile([128, SHARD + 8, 2], f16, tag="tab",
                                      bufs=1)
                        tab_builder(h, tab, mp)
                        if h == 1:
                            emit_cc(0)
                        pending = []
                        hcalls = [c for c in calls if c["h"] == h]
                        for cl in hcalls:
                            ni, nip = cl["ni"], cl["nip"]
                            ixt = mp.tile([128, cap // 16], i16, tag="gix",
                                          bufs=2)
                            nc.sync.dma_start(
                                out=ixt[:, :nip // 16],
                                in_=gidx[:, cl["off"]:cl["off"] + nip // 16])
                            msg = mp.tile([128, cap, 2], f16, tag="msg",
                                          bufs=2)
                            nc.gpsimd.ap_gather(
                                msg[:, :nip, :].bitcast(f32),
                                tab[:].bitcast(f32)[:, :SHARD + 8 * (L == 2)],
                                ixt[:, :nip // 16],
                                channels=128,
                                num_elems=SHARD + 8 * (L == 2), d=1,
                                num_idxs=nip)
                            off = 0
                            for (o, g0, nn, K, a0, first) in cl["segs"]:
                                if o not in acc_t:
                                    acc_t[o] = mp.tile([128, SPC, 2], f16, name=f"acc{o}",
                                                       tag="acc", bufs=2)
                                acc = acc_t[o]
                                if L == 1:
                                    v = msg[:, off:off + nn * K, :].rearrange(
                                        "p (n k) j -> p n k j", k=K)
                                    dstv = acc[:, a0:a0 + nn, :]
                                else:
                                    v = msg[:, off:off + nn * K, :].bitcast(
                                        f32).rearrange(
                                        "p (n k) u -> p n k u", k=K)
                                    dstv = acc[:].bitcast(f32)[:, a0:a0 + nn]
                                if K == 1:
                                    nc.vector.tensor_copy(dstv, v[:, :, 0, :])
                                else:
                                    _tree_reduce(nc, v, K, dstv)
                                off += nn * K
                            todo = pending
                            pending = cl["done"]
                            if cl is hcalls[-1]:
                                todo = todo + pending
                                pending = []
                            for o in todo:
                                acc = acc_t.pop(o)
                                for (z0, zn) in zero_rngs[L][(h, o)]:
                                    nc.vector.memset(acc[:, z0:z0 + zn, :], 0)
                                S = mp.tile([128, SPC, 2], f16, tag="S",
                                            bufs=1)
                                bi = (L - 1) * 2 + h
                                nc.gpsimd.ap_gather(
                                    S[:].bitcast(f32), acc[:].bitcast(f32),
                                    uix[:, (bi * NC + o) * (SPC // 16):
                                        (bi * NC + o + 1) * (SPC // 16)],
                                    channels=128, num_elems=SPC, d=1,
                                    num_idxs=SPC)
                                if L == 1:
                                    nc.sync.dma_start(
                                        out=part[h][o].rearrange(
                                            "p m j -> p (m j)"),
                                        in_=S[:].rearrange(
                                            "p m j -> p (m j)"))
                                else:
                                    nc.sync.dma_start(
                                        out=part[h][o],
                                        in_=S[:].bitcast(f32))
                    emit_cc(1)

            def tab1_builder(h, tab, mp):
                for q0 in range(0, SHARD, SPC):
                    xs = mp.tile([F_IN, SPC], f16, tag="xsrc", bufs=2)
                    nc.sync.dma_start(out=xs[:],
                                      in_=xt[:, h * SHARD + q0:
                                             h * SHARD + q0 + SPC])
                    for c0 in range(0, SPC, 512):
                        cw = min(512, SPC - c0)
                        for j, wt in ((0, w1at), (1, w1bt)):
                            ps = psp.tile([128, 512], f32, tag="ps")
                            nc.tensor.matmul(ps[:, :cw], lhsT=wt[:],
                                             rhs=xs[:, c0:c0 + cw],
                                             start=True, stop=True)
                            if j == 0:
                                nc.scalar.activation(
                                    tab[:, q0 + c0:q0 + c0 + cw, j],
                                    ps[:, :cw], AF.Copy)
                            else:
                                nc.vector.tensor_copy(
                                    tab[:, q0 + c0:q0 + c0 + cw, j],
                                    ps[:, :cw])

            def tab2_builder(h, tab, mp):
                CW = 512
                HM = SPC // 2                    # 1568 m-positions per shard
                tabv = tab[:].bitcast(f32).rearrange("p (m s) u -> p m (s u)", s=8)
                nc.vector.memset(tabv[:, HM, :], 0)   # zero pad cols
                for b in range(4):
                    p0 = 32 * b
                    dvh = mp.tile([32, SPC // 2], f16, tag="dsq", bufs=2)
                    nc.sync.dma_start(
                        out=dvh[:],
                        in_=dvt[p0:p0 + 32,
                                (SPC // 2) * h:(SPC // 2) * (h + 1)])
                    for c0 in range(0, HM, CW):
                        cw = min(CW, HM - c0)
                        cm = HM * h + c0
                        t0 = mp.tile([32, CW, 2], f16, tag="h1c", bufs=2)
                        nc.sync.dma_start(
                            out=t0[:, :cw, :],
                            in_=rs1[0][p0:p0 + 32, cm:cm + cw, :])
                        t1 = mp.tile([32, CW, 2], f16, tag="h1d", bufs=2)
                        nc.sync.dma_start(
                            out=t1[:, :cw, :],
                            in_=rs1[1][p0:p0 + 32, cm:cm + cw, :])
                        nc.vector.tensor_add(out=t0[:, :cw, :],
                                             in0=t0[:, :cw, :],
                                             in1=t1[:, :cw, :])
                        dvb = dvh[:, c0:c0 + cw].broadcast_to([32, cw, 2])
                        nc.vector.tensor_mul(out=t0[:, :cw, :],
                                             in0=t0[:, :cw, :], in1=dvb)
                        if not b1z:
                            bb = mp.tile([32, CW, 2], f16, tag="b1c", bufs=2)
                            nc.sync.dma_start(
                                out=bb[:, :cw, :],
                                in_=b1big[p0:p0 + 32, cm:cm + cw, :])
                            nc.vector.tensor_add(out=t0[:, :cw, :],
                                                 in0=t0[:, :cw, :],
                                                 in1=bb[:, :cw, :])
                        nc.scalar.activation(t0[:, :cw, :], t0[:, :cw, :],
                                             AF.Relu)
                        if not b1z:
                            nc.vector.tensor_mul(out=t0[:, :cw, :],
                                                 in0=t0[:, :cw, :], in1=dvb)
                        for par in (0, 1):
                            u = 2 * b + par
                            ps = psp.tile([128, CW], f32, tag="ps")
                            nc.tensor.matmul(
                                ps[:, :cw], lhsT=w2vt[2 * par][:],
                                rhs=t0[:, :cw, 0],
                                start=True, stop=False)
                            nc.tensor.matmul(
                                ps[:, :cw], lhsT=w2vt[2 * par + 1][:],
                                rhs=t0[:, :cw, 1],
                                start=False, stop=True)
                            nc.scalar.activation(
                                tabv[:, c0:c0 + cw, u], ps[:, :cw], AF.Copy)

            run_layer(1, calls1, gidx1, part1, rs1, NI_CAP1, tab1_builder)
            run_layer(2, calls2, gidx2, part2, rs2, NI_CAP2, tab2_builder)

            with tc.tile_pool(name="fin", bufs=1) as fpool:
                dv2 = fpool.tile([128, SPC], f32, tag="dv2")
                nc.sync.dma_start(out=dv2[:], in_=dinv2[:])
                o0 = fpool.tile([128, SPC, 2], f16, tag="fo")
                nc.sync.dma_start(out=o0[:].bitcast(f32), in_=rs2[0][:])
                nc.vector.tensor_mul(out=o0[:].bitcast(f32),
                                     in0=o0[:].bitcast(f32), in1=dv2[:])
                nc.vector.tensor_scalar_add(o0[:].bitcast(f32),
                                            o0[:].bitcast(f32), b2t[:])
                o1 = fpool.tile([128, SPC, 2], f16, tag="fo2")
                HF = SPC // 2
                for z in (0, 1):
                    sl = slice(z * HF, (z + 1) * HF)
                    nc.sync.dma_start(out=o1[:].bitcast(f32)[:, sl],
                                      in_=rs2[1][:, sl])
                    nc.vector.tensor_mul(out=o1[:].bitcast(f32)[:, sl],
                                         in0=o1[:].bitcast(f32)[:, sl],
                                         in1=dv2[:, sl])
                    nc.vector.tensor_add(out=o0[:].bitcast(f32)[:, sl],
                                         in0=o0[:].bitcast(f32)[:, sl],
                                         in1=o1[:].bitcast(f32)[:, sl])
                    nc.sync.dma_start(out=out[:, sl],
                                      in_=o0[:].bitcast(f32)[:, sl])
    nc.compile()
    return nc


# ----------------------------------------------------------------- wrapper
_CACHE = {}


def kernel(x, edge_index, W1, b1, W2, b2):
    in_maps, schedule, meta = prepare(x, edge_index, W1, b1, W2, b2)
    key = (schedule["K1"].tobytes() + schedule["K2"].tobytes() + bytes([schedule["b1z"]]))
    if key not in _CACHE:
        _CACHE[key] = build(schedule)
    nc = _CACHE[key]
    res = run_bass_kernel_spmd(nc, in_maps, list(range(NC)))
    scol = meta["scol_of_local"]
    full = np.empty((N, F_OUT), np.float32)
    for c in range(NC):
        outc = np.asarray(res.results[c]["out"])     # [128, SPC]
        sc = scol[c, :SL]
        s, m = sc // SPC, sc % SPC
        full[c * SL:(c + 1) * SL] = outc[
            (16 * s[:, None] + np.arange(F_OUT)[None, :]), m[:, None]]
    return full



# revision 23
# speedup vs baseline: 1.0970x; 1.0149x over previous
"""GCN 2-layer encoder on 8 Trainium2 NeuronCores (Bass/Tile), v2.

Push-sharded: core c owns src slice [25000c, 25000(c+1)).  Per layer, each
core builds a feature-major table T^T = (h*dinv) @ W in SBUF (f16 pairs
packed as f32 for L1, f32 for L2), then aggregates messages for ALL dst
nodes with GPSIMD ap_gather (SBUF->SBUF, 8 idx streams, d=1 f32) over a
host-built ELL schedule, tree-adds (DVE) reduce each dst's K slots, partial
sums accumulate per-owner in SBUF, a per-stream ap_gather unpermutes from
per-core degree-sorted rank order to a shared order, and ReduceScatter
(one per src-shard pass, overlapped) sums partials across cores.
Final output is assembled (unpermuted/transposed) on host.
"""
import numpy as np

import concourse.bass as bass
import concourse.bacc as bacc
import concourse.mybir as mybir
import concourse.tile as tile
from concourse.bass_utils import run_bass_kernel_spmd

N = 200000
E = 6400000
F_IN, F_HID, F_OUT = 32, 32, 16
NC = 8
SL = N // NC            # 25000 real nodes per owner
LP = 25088              # padded slice (196*128)
NSTR = 8                # idx streams (16-partition groups)
SPC = LP // NSTR        # 3136 shared cols per (owner, stream)
GRP = 64                # ELL rank-group granularity
NGO = SPC // GRP        # 49 rank groups per stream
RSH = SL // 2           # 12500: L1 shard split on raw local ids
SHARD = LP // 2         # 12544: table columns per pass
NI_CAP1 = 11776         # max gather idxs per call, layer 1 (f16-pair msgs)
NI_CAP2 = 11520         # layer 2 (f32 msgs)

f32, f16, i16 = mybir.dt.float32, mybir.dt.float16, mybir.dt.int16
AF = mybir.ActivationFunctionType
LOCALITY = True          # sort gather idx for SBUF-read locality


# ----------------------------------------------------------------- host prep
def _make_calls(K, cap):
    """K: [2, NC, NGO] -> list of calls (pass-major, owner asc, group asc).
    call = dict(h, ni, off, segs=[(owner, g0, nn, K, acc_off, first)],
    done=[o...]). Segments merge adjacent equal-K full groups of one owner."""
    calls = []
    for h in (0, 1):
        # flat list of (owner, acc_off, nnodes, K) units; split nodes freely
        units = []
        for o in range(NC):
            for g in range(NGO):
                kk = int(K[h, o, g])
                if kk == 0:
                    continue
                if units and units[-1][0] == o and units[-1][3] == kk \
                        and units[-1][1] + units[-1][2] == GRP * g:
                    o0, a0, nn, k0 = units[-1]
                    units[-1] = (o0, a0, nn + GRP, k0)
                else:
                    units.append((o, GRP * g, GRP, kk))
        packs, cur, w = [], [], 0
        ui = 0
        cap_use = cap - 15
        units = [list(u) for u in units]
        while ui < len(units):
            o, a0, nn, kk = units[ui]
            room = (cap_use - w) // kk
            if room < 1:
                packs.append(cur)
                cur, w = [], 0
                continue
            take = min(nn, room)
            cur.append((o, a0, take, kk))
            w += take * kk
            if take == nn:
                ui += 1
            else:
                units[ui] = [o, a0 + take, nn - take, kk]
        if cur:
            packs.append(cur)
        last_pack_of_owner = {}
        for pi, pk in enumerate(packs):
            for (o, a0, nn, kk) in pk:
                last_pack_of_owner[o] = pi
        for pi, pk in enumerate(packs):
            segs = [(o, a0 // GRP, nn, kk, a0, True) for (o, a0, nn, kk) in pk]
            done = [o for o, pi2 in last_pack_of_owner.items() if pi2 == pi]
            ni = sum(s[2] * s[3] for s in segs)
            nip = (ni + 15) // 16 * 16
            calls.append(dict(h=h, ni=ni, nip=nip, segs=segs,
                              done=sorted(done)))
    off = 0
    for cl in calls:
        cl["off"] = off
        off += cl["nip"] // 16
    return calls


def prepare(x, edge_index, W1, b1, W2, b2):
    x = np.asarray(x, np.float32)
    src = np.asarray(edge_index[0], dtype=np.int64)
    dst = np.asarray(edge_index[1], dtype=np.int64)
    W1 = np.asarray(W1, np.float32)
    W2 = np.asarray(W2, np.float32)
    b1 = np.asarray(b1, np.float32)
    b2 = np.asarray(b2, np.float32)

    deg = (np.bincount(dst, minlength=N) + 1).astype(np.float64)
    dinv = (1.0 / np.sqrt(deg)).astype(np.float32)

    # shared stream/col assignment per owner (snake-deal by total deg desc)
    scol_of_local = np.empty((NC, LP), np.int64)
    for o in range(NC):
        cnt = np.zeros(LP, np.int64)
        cnt[:SL] = deg[o * SL:(o + 1) * SL]
        order = np.argsort(-cnt, kind="stable")
        sc = (np.arange(LP) % NSTR) * SPC + (np.arange(LP) // NSTR)
        scol_of_local[o, order] = sc
    PAD1 = SHARD - 1                            # xt col 12543 is zero
    PAD2 = SHARD                                # explicit zero col in tab2

    K1 = np.zeros((2, NC, NGO), np.int64)
    K2 = np.zeros((2, NC, NGO), np.int64)
    so = src // SL
    eorder = np.argsort(so, kind="stable")
    bounds = np.searchsorted(so[eorder], np.arange(NC + 1))

    cores = []
    for c in range(NC):
        e = eorder[bounds[c]:bounds[c + 1]]
        s_loc = np.concatenate([src[e] - SL * c, np.arange(SL, dtype=np.int64)])
        d_glob = np.concatenate([dst[e], np.arange(SL, dtype=np.int64) + SL * c])
        h1p = s_loc // RSH
        col1 = s_loc - RSH * h1p
        sc_src = scol_of_local[c, s_loc]
        c2 = (sc_src % SPC) * NSTR + sc_src // SPC   # m-major column id
        h2p = c2 // SHARD
        col2 = c2 - SHARD * h2p
        do = d_glob // SL
        dl = d_glob - SL * do
        dsc = scol_of_local[do, dl]
        seg = do * NSTR + dsc // SPC            # (owner, stream) 0..63
        dm = dsc - (dsc // SPC) * SPC
        gkey = seg * SPC + dm                   # 0..200703

        # per (layer, pass) private rank order: sort by per-pass count desc,
        # then within each GRP-rank block reorder by min gathered column so
        # gather idx streams are locally ascending (SBUF-read locality).
        drs = {}
        rank_ofs = {}
        for (L, K, hp, colv) in ((1, K1, h1p, col1), (2, K2, h2p, col2)):
            dr = np.empty(len(s_loc), np.int64)
            for h in (0, 1):
                sel = hp == h
                chp = np.bincount(gkey[sel],
                                  minlength=NC * LP).reshape(NC * NSTR, SPC)
                rk = np.argsort(-chp, axis=1, kind="stable")
                cr = np.take_along_axis(chp, rk, axis=1)
                gmax = cr.reshape(NC, NSTR, NGO, GRP).max(axis=(1, 3))
                np.maximum(K[h], gmax, out=K[h])
                if LOCALITY:
                    # min column per (seg, dm)
                    key0 = gkey[sel]
                    cs = colv[sel]
                    srt = np.argsort(key0 * (1 << 16) + cs, kind="stable")
                    ks, vs = key0[srt], cs[srt]
                    first = np.r_[True, ks[1:] != ks[:-1]]
                    minc = np.full(NC * LP, 1 << 20, np.int64)
                    minc[ks[first]] = vs[first]
                    minc = minc.reshape(NC * NSTR, SPC)
                    vals = np.take_along_axis(minc, rk, axis=1)
                    ord_in = np.argsort(
                        vals.reshape(NC * NSTR, NGO, GRP), axis=2,
                        kind="stable")
                    rk = np.take_along_axis(
                        rk.reshape(NC * NSTR, NGO, GRP), ord_in,
                        axis=2).reshape(NC * NSTR, SPC)
                rank_of = np.empty_like(rk)
                np.put_along_axis(rank_of, rk, np.broadcast_to(
                    np.arange(SPC)[None, :], rk.shape).copy(), axis=1)
                dr[sel] = rank_of[seg[sel], dm[sel]]
                rank_ofs[(L, h)] = rank_of
            drs[L] = dr
        cores.append(dict(h1p=h1p, col1=col1, h2p=h2p, col2=col2,
                          seg=seg, dr1=drs[1], dr2=drs[2],
                          rank_ofs=rank_ofs))

    calls1 = _make_calls(K1, NI_CAP1)
    calls2 = _make_calls(K2, NI_CAP2)
    C1 = sum(cl["nip"] for cl in calls1) // 16
    C2 = sum(cl["nip"] for cl in calls2) // 16
    zero_rngs = {1: {}, 2: {}}
    for L, K in ((1, K1), (2, K2)):
        for h in (0, 1):
            for o in range(NC):
                rng = []
                for g in range(NGO):
                    if K[h, o, g] == 0:
                        if rng and rng[-1][0] + rng[-1][1] == GRP * g:
                            rng[-1] = (rng[-1][0], rng[-1][1] + GRP)
                        else:
                            rng.append((GRP * g, GRP))
                zero_rngs[L][(h, o)] = rng
    schedule = dict(K1=K1, K2=K2, calls1=calls1, calls2=calls2,
                    C1=C1, C2=C2, zero_rngs=zero_rngs,
                    b1z=bool(not np.any(b1)))

    # ---------------- per-core runtime data
    in_maps = []
    for c in range(NC):
        cc = cores[c]

        def build_gidx(hp, colv, calls, padcols, dr):
            key = (hp * (NC * LP) + cc["seg"] * SPC + dr).astype(np.int64)
            o2 = np.lexsort((colv, key)) if LOCALITY \
                else np.argsort(key, kind="stable")
            cols_sorted = colv[o2]
            cnt = np.bincount(key, minlength=2 * NC * LP)
            ptr = np.zeros(2 * NC * LP + 1, np.int64)
            np.cumsum(cnt, out=ptr[1:])
            tiles = []
            for cl in calls:
                h, ni, nip = cl["h"], cl["ni"], cl["nip"]
                til = np.empty((128, nip // 16), np.int16)
                for s in range(NSTR):
                    parts = []
                    for (o, g0, nn, K, a0, first) in cl["segs"]:
                        base = h * (NC * LP) + (o * NSTR + s) * SPC
                        pos = base + a0 + np.arange(nn)
                        take = ptr[pos][:, None] + np.arange(K)[None, :]
                        valid = np.arange(K)[None, :] < cnt[pos][:, None]
                        vals = np.where(
                            valid,
                            cols_sorted[np.minimum(take, len(cols_sorted) - 1)],
                            padcols[h])
                        parts.append(vals.ravel())
                    parts.append(np.full(nip - ni, padcols[h], np.int64))
                    sv = np.concatenate(parts)
                    til[16 * s:16 * s + 16] = sv.reshape(nip // 16, 16).T
                tiles.append(til)
            return np.concatenate(tiles, axis=1)

        gidx1 = build_gidx(cc["h1p"], cc["col1"], calls1, (PAD1, PAD1),
                           cc["dr1"])
        gidx2 = build_gidx(cc["h2p"], cc["col2"], calls2, (PAD2, PAD2),
                           cc["dr2"])

        # uidx blocks ordered (L1A, L1B, L2A, L2B), each [128, NC*196]
        uidx = np.empty((128, 4 * NC * (SPC // 16)), np.int16)
        for bi, (L, h) in enumerate(((1, 0), (1, 1), (2, 0), (2, 1))):
            ro = cc["rank_ofs"][(L, h)]
            for o in range(NC):
                for s in range(NSTR):
                    r = ro[o * NSTR + s]             # [m] -> rank
                    uidx[16 * s:16 * s + 16,
                         (bi * NC + o) * (SPC // 16):
                         (bi * NC + o + 1) * (SPC // 16)] = \
                        r.reshape(SPC // 16, 16).T
        xt = np.zeros((F_IN, 2 * SHARD), np.float16)
        xs = (x[c * SL:(c + 1) * SL] * dinv[c * SL:(c + 1) * SL, None]).T
        xt[:, :RSH] = xs[:, :RSH]
        xt[:, SHARD:SHARD + RSH] = xs[:, RSH:]
        d_ord = np.ones(LP, np.float32)
        loc = np.argsort(scol_of_local[c])          # scol -> local id
        real = loc < SL
        d_ord[real] = dinv[c * SL + loc[real]]
        dinv2 = np.empty((128, SPC), np.float32)
        for s in range(NSTR):
            dinv2[16 * s:16 * s + 16] = d_ord[s * SPC:(s + 1) * SPC][None, :]
        b1z = not np.any(b1)
        # pre-table scale tile in pair layout: dinv^2 if b1==0 else dinv
        dvt = (dinv2 * dinv2 if b1z else dinv2).astype(np.float16)
        fp = np.arange(128) % 16
        w1a = W1[:, 2 * fp].astype(np.float16)
        w1b = W1[:, 2 * fp + 1].astype(np.float16)
        # w2v[q]: q=2*parity+j, nonzero rows [16*parity,16*parity+16):
        #   row 16*parity+fp = W2[2fp+j, col%16]
        w2v = np.zeros((4, 32, 128), np.float16)
        for par in (0, 1):
            for j in (0, 1):
                q = 2 * par + j
                for fpp in range(16):
                    w2v[q, 16 * par + fpp] = W2[2 * fpp + j][
                        np.arange(128) % 16]
        b2P = np.tile(b2, 8).reshape(128, 1).astype(np.float32)
        blobB = np.concatenate(
            [xt, w1a, w1b] + [w2v[q] for q in range(4)], axis=1)
        blobBp = np.zeros((128, blobB.shape[1]), np.float16)
        blobBp[:F_IN] = blobB
        blobA = np.concatenate(
            [np.ascontiguousarray(dinv2).view(np.int16),
             np.ascontiguousarray(b2P).view(np.int16),
             uidx, gidx1, gidx2,
             dvt.view(np.int16),
             blobBp.view(np.int16)], axis=1)
        if blobA.shape[1] % 2:
            blobA = np.concatenate(
                [blobA, np.zeros((128, 1), np.int16)], axis=1)
        im = {"blobA": blobA}
        if not b1z:
            b1big = np.empty((128, SPC, 2), np.float16)
            for j in (0, 1):
                b1big[:, :, j] = b1[2 * (np.arange(128)[:, None] % 16) + j]
            im["b1big"] = b1big
        in_maps.append(im)
    meta = dict(scol_of_local=scol_of_local)
    return in_maps, schedule, meta


# ----------------------------------------------------------------- build
def _tree_reduce(nc, v, K, final_out):
    """Sum the k axis of v [p, nn, K, j]; the last add writes final_out."""
    ops = []
    k = K
    while k > 1:
        if k % 2 == 1:
            ops.append((0, 1, k - 1, k))
            k -= 1
        half = k // 2
        ops.append((0, half, half, k))
        k = half
    for i, (o0, o1, i0, i1) in enumerate(ops):
        a = v[:, :, o0:o1, :]
        b = v[:, :, i0:i1, :]
        if i == len(ops) - 1:
            nc.vector.tensor_add(out=final_out, in0=a, in1=b)
        else:
            nc.vector.tensor_add(out=a, in0=a, in1=b)


def build(schedule):
    calls1, calls2 = schedule["calls1"], schedule["calls2"]
    C1, C2 = schedule["C1"], schedule["C2"]
    zero_rngs = schedule["zero_rngs"]

    nc = bacc.Bacc("TRN2", target_bir_lowering=False, debug=False,
                   num_devices=NC)
    UW = 4 * NC * (SPC // 16)
    WB = 2 * SHARD + 128 + 128 + 4 * 128
    WA = UW + C1 + C2 + SPC + 2 * SPC + 2 + WB
    WA += WA % 2
    blobA = nc.dram_tensor("blobA", [128, WA], i16, kind="ExternalInput").ap()
    off = 0
    dinv2 = blobA[:, off:off + 2 * SPC].bitcast(f32); off += 2 * SPC
    b2P = blobA[:, off:off + 2].bitcast(f32); off += 2
    uidx = blobA[:, off:off + UW]; off += UW
    gidx1 = blobA[:, off:off + C1]; off += C1
    gidx2 = blobA[:, off:off + C2]; off += C2
    dvt = blobA[:, off:off + SPC].bitcast(f16); off += SPC
    blobB = blobA[:F_IN, off:].bitcast(f16)
    off = 0
    xt = blobB[:, off:off + 2 * SHARD]; off += 2 * SHARD
    w1a = blobB[:, off:off + 128]; off += 128
    w1b = blobB[:, off:off + 128]; off += 128
    w2v = [blobB[:, off + q * 128:off + (q + 1) * 128] for q in range(4)]
    b1z = schedule["b1z"]
    b1big = (None if b1z else nc.dram_tensor(
        "b1big", [128, SPC, 2], f16, kind="ExternalInput").ap())
    out = nc.dram_tensor("out", [128, SPC], f32, kind="ExternalOutput").ap()

    part1 = [nc.dram_tensor(f"part1{h}", [NC, 128, SPC, 2], f16).ap()
             for h in (0, 1)]
    rs1 = [nc.dram_tensor(f"rs1{h}", [128, SPC, 2], f16).ap() for h in (0, 1)]
    part2 = [nc.dram_tensor(f"part2{h}", [NC, 128, SPC], f32).ap()
             for h in (0, 1)]
    rs2 = [nc.dram_tensor(f"rs2{h}", [128, SPC], f32).ap() for h in (0, 1)]
    HM = SPC // 2
    # h=1 ReduceScatters are split into column halves (separate tensors) so
    # downstream consumers of the first half can start while the second half
    # is still on the fabric.
    rs1h = [nc.dram_tensor(f"rs1h{z}", [128, HM, 2], f16).ap()
            for z in (0, 1)]
    rs2h = [nc.dram_tensor(f"rs2h{z}", [128, HM], f32).ap() for z in (0, 1)]
    part1h = [nc.dram_tensor(f"part1h{z}", [NC, 128, HM, 2], f16).ap()
              for z in (0, 1)]
    part2h = [nc.dram_tensor(f"part2h{z}", [NC, 128, HM], f32).ap()
              for z in (0, 1)]

    with tile.TileContext(nc) as tc:
        with tc.tile_pool(name="const", bufs=1) as const, \
             tc.tile_pool(name="psum", bufs=4, space="PSUM") as psp:
            w1at = const.tile([F_IN, 128], f16)
            nc.sync.dma_start(out=w1at[:], in_=w1a[:])
            w1bt = const.tile([F_IN, 128], f16)
            nc.sync.dma_start(out=w1bt[:], in_=w1b[:])
            w2vt = []
            for q in range(4):
                wv = const.tile([32, 128], f16, name=f"w2v{q}", tag=f"w2v{q}")
                nc.sync.dma_start(out=wv[:], in_=w2v[q])
                w2vt.append(wv)
            b2t = const.tile([128, 1], f32)
            nc.sync.dma_start(out=b2t[:], in_=b2P[:])
            uix = const.tile([128, 4 * NC * (SPC // 16)], i16)
            nc.sync.dma_start(out=uix[:], in_=uidx[:])

            def run_layer(L, calls, gidx, part, parth, rsl, rshl, cap,
                          tab_builder):
                acc_t = {}
                def emit_cc(h):
                    if h == 0:
                        bass.BassGpSimd.collective_compute(
                            nc.gpsimd, "ReduceScatter", mybir.AluOpType.add,
                            replica_groups=[list(range(NC))],
                            ins=[part[h][:]], outs=[rsl[h][:]])
                    else:
                        for z in (0, 1):
                            bass.BassGpSimd.collective_compute(
                                nc.gpsimd, "ReduceScatter",
                                mybir.AluOpType.add,
                                replica_groups=[list(range(NC))],
                                ins=[parth[z][:]], outs=[rshl[z][:]])

                with tc.tile_pool(name=f"w{L}", bufs=1) as wp, \
                     tc.tile_pool(name=f"m{L}", bufs=2) as mp:
                    for h in (0, 1):
                        tab = wp.tile([128, SHARD + 8, 2], f16, tag="tab",
                                      bufs=1)
                        tab_builder(h, tab, mp)
                        if h == 1:
                            emit_cc(0)
                        pending = []
                        hcalls = [c for c in calls if c["h"] == h]
                        for cl in hcalls:
                            ni, nip = cl["ni"], cl["nip"]
                            ixt = mp.tile([128, cap // 16], i16, tag="gix",
                                          bufs=2)
                            nc.sync.dma_start(
                                out=ixt[:, :nip // 16],
                                in_=gidx[:, cl["off"]:cl["off"] + nip // 16])
                            msg = mp.tile([128, cap, 2], f16, tag="msg",
                                          bufs=2)
                            nc.gpsimd.ap_gather(
                                msg[:, :nip, :].bitcast(f32),
                                tab[:].bitcast(f32)[:, :SHARD + 8 * (L == 2)],
                                ixt[:, :nip // 16],
                                channels=128,
                                num_elems=SHARD + 8 * (L == 2), d=1,
                                num_idxs=nip)
                            off = 0
                            for (o, g0, nn, K, a0, first) in cl["segs"]:
                                if o not in acc_t:
                                    acc_t[o] = mp.tile([128, SPC, 2], f16, name=f"acc{o}",
                                                       tag="acc", bufs=2)
                                acc = acc_t[o]
                                if L == 1:
                                    v = msg[:, off:off + nn * K, :].rearrange(
                                        "p (n k) j -> p n k j", k=K)
                                    dstv = acc[:, a0:a0 + nn, :]
                                else:
                                    v = msg[:, off:off + nn * K, :].bitcast(
                                        f32).rearrange(
                                        "p (n k) u -> p n k u", k=K)
                                    dstv = acc[:].bitcast(f32)[:, a0:a0 + nn]
                                if K == 1:
                                    nc.vector.tensor_copy(dstv, v[:, :, 0, :])
                                else:
                                    _tree_reduce(nc, v, K, dstv)
                                off += nn * K
                            todo = pending
                            pending = cl["done"]
                            if cl is hcalls[-1]:
                                todo = todo + pending
                                pending = []
                            for o in todo:
                                acc = acc_t.pop(o)
                                for (z0, zn) in zero_rngs[L][(h, o)]:
                                    nc.vector.memset(acc[:, z0:z0 + zn, :], 0)
                                S = mp.tile([128, SPC, 2], f16, tag="S",
                                            bufs=1)
                                bi = (L - 1) * 2 + h
                                nc.gpsimd.ap_gather(
                                    S[:].bitcast(f32), acc[:].bitcast(f32),
                                    uix[:, (bi * NC + o) * (SPC // 16):
                                        (bi * NC + o + 1) * (SPC // 16)],
                                    channels=128, num_elems=SPC, d=1,
                                    num_idxs=SPC)
                                if h == 0:
                                    if L == 1:
                                        nc.sync.dma_start(
                                            out=part[h][o].rearrange(
                                                "p m j -> p (m j)"),
                                            in_=S[:].rearrange(
                                                "p m j -> p (m j)"))
                                    else:
                                        nc.sync.dma_start(
                                            out=part[h][o],
                                            in_=S[:].bitcast(f32))
                                else:
                                    for z in (0, 1):
                                        sz = S[:, z * HM:(z + 1) * HM, :]
                                        if L == 1:
                                            nc.sync.dma_start(
                                                out=parth[z][o].rearrange(
                                                    "p m j -> p (m j)"),
                                                in_=sz.rearrange(
                                                    "p m j -> p (m j)"))
                                        else:
                                            nc.sync.dma_start(
                                                out=parth[z][o],
                                                in_=sz.bitcast(f32))
                    emit_cc(1)

            def tab1_builder(h, tab, mp):
                for q0 in range(0, SHARD, SPC):
                    xs = mp.tile([F_IN, SPC], f16, tag="xsrc", bufs=2)
                    nc.sync.dma_start(out=xs[:],
                                      in_=xt[:, h * SHARD + q0:
                                             h * SHARD + q0 + SPC])
                    for c0 in range(0, SPC, 512):
                        cw = min(512, SPC - c0)
                        for j, wt in ((0, w1at), (1, w1bt)):
                            ps = psp.tile([128, 512], f32, tag="ps")
                            nc.tensor.matmul(ps[:, :cw], lhsT=wt[:],
                                             rhs=xs[:, c0:c0 + cw],
                                             start=True, stop=True)
                            if j == 0:
                                nc.scalar.activation(
                                    tab[:, q0 + c0:q0 + c0 + cw, j],
                                    ps[:, :cw], AF.Copy)
                            else:
                                nc.vector.tensor_copy(
                                    tab[:, q0 + c0:q0 + c0 + cw, j],
                                    ps[:, :cw])

            def tab2_builder(h, tab, mp):
                CW = 512
                HM = SPC // 2                    # 1568 m-positions per shard
                tabv = tab[:].bitcast(f32).rearrange("p (m s) u -> p m (s u)", s=8)
                nc.vector.memset(tabv[:, HM, :], 0)   # zero pad cols
                for b in range(4):
                    p0 = 32 * b
                    dvh = mp.tile([32, SPC // 2], f16, tag="dsq", bufs=2)
                    nc.sync.dma_start(
                        out=dvh[:],
                        in_=dvt[p0:p0 + 32,
                                (SPC // 2) * h:(SPC // 2) * (h + 1)])
                    for c0 in range(0, HM, CW):
                        cw = min(CW, HM - c0)
                        cm = HM * h + c0
                        t0 = mp.tile([32, CW, 2], f16, tag="h1c", bufs=2)
                        nc.sync.dma_start(
                            out=t0[:, :cw, :],
                            in_=rs1[0][p0:p0 + 32, cm:cm + cw, :])
                        t1 = mp.tile([32, CW, 2], f16, tag="h1d", bufs=2)
                        nc.sync.dma_start(
                            out=t1[:, :cw, :],
                            in_=rs1h[h][p0:p0 + 32, c0:c0 + cw, :])
                        nc.vector.tensor_add(out=t0[:, :cw, :],
                                             in0=t0[:, :cw, :],
                                             in1=t1[:, :cw, :])
                        dvb = dvh[:, c0:c0 + cw].broadcast_to([32, cw, 2])
                        nc.vector.tensor_mul(out=t0[:, :cw, :],
                                             in0=t0[:, :cw, :], in1=dvb)
                        if not b1z:
                            bb = mp.tile([32, CW, 2], f16, tag="b1c", bufs=2)
                            nc.sync.dma_start(
                                out=bb[:, :cw, :],
                                in_=b1big[p0:p0 + 32, cm:cm + cw, :])
                            nc.vector.tensor_add(out=t0[:, :cw, :],
                                                 in0=t0[:, :cw, :],
                                                 in1=bb[:, :cw, :])
                        nc.scalar.activation(t0[:, :cw, :], t0[:, :cw, :],
                                             AF.Relu)
                        if not b1z:
                            nc.vector.tensor_mul(out=t0[:, :cw, :],
                                                 in0=t0[:, :cw, :], in1=dvb)
                        for par in (0, 1):
                            u = 2 * b + par
                            ps = psp.tile([128, CW], f32, tag="ps")
                            nc.tensor.matmul(
                                ps[:, :cw], lhsT=w2vt[2 * par][:],
                                rhs=t0[:, :cw, 0],
                                start=True, stop=False)
                            nc.tensor.matmul(
                                ps[:, :cw], lhsT=w2vt[2 * par + 1][:],
                                rhs=t0[:, :cw, 1],
                                start=False, stop=True)
                            nc.scalar.activation(
                                tabv[:, c0:c0 + cw, u], ps[:, :cw], AF.Copy)

            run_layer(1, calls1, gidx1, part1, part1h, rs1, rs1h, NI_CAP1,
                      tab1_builder)
            run_layer(2, calls2, gidx2, part2, part2h, rs2, rs2h, NI_CAP2,
                      tab2_builder)

            with tc.tile_pool(name="fin", bufs=1) as fpool:
                dv2 = fpool.tile([128, SPC], f32, tag="dv2")
                nc.sync.dma_start(out=dv2[:], in_=dinv2[:])
                o0 = fpool.tile([128, SPC, 2], f16, tag="fo")
                nc.sync.dma_start(out=o0[:].bitcast(f32), in_=rs2[0][:])
                nc.vector.tensor_mul(out=o0[:].bitcast(f32),
                                     in0=o0[:].bitcast(f32), in1=dv2[:])
                nc.vector.tensor_scalar_add(o0[:].bitcast(f32),
                                            o0[:].bitcast(f32), b2t[:])
                o1 = fpool.tile([128, SPC, 2], f16, tag="fo2")
                HF = SPC // 2
                for z in (0, 1):
                    sl = slice(z * HF, (z + 1) * HF)
                    nc.sync.dma_start(out=o1[:].bitcast(f32)[:, sl],
                                      in_=rs2h[z][:])
                    nc.vector.tensor_mul(out=o1[:].bitcast(f32)[:, sl],
                                         in0=o1[:].bitcast(f32)[:, sl],
                                         in1=dv2[:, sl])
                    nc.vector.tensor_add(out=o0[:].bitcast(f32)[:, sl],
                                         in0=o0[:].bitcast(f32)[:, sl],
                                         in1=o1[:].bitcast(f32)[:, sl])
                    nc.sync.dma_start(out=out[:, sl],
                                      in_=o0[:].bitcast(f32)[:, sl])
    nc.compile()
    return nc


# ----------------------------------------------------------------- wrapper
_CACHE = {}


def kernel(x, edge_index, W1, b1, W2, b2):
    in_maps, schedule, meta = prepare(x, edge_index, W1, b1, W2, b2)
    key = (schedule["K1"].tobytes() + schedule["K2"].tobytes() + bytes([schedule["b1z"]]))
    if key not in _CACHE:
        _CACHE[key] = build(schedule)
    nc = _CACHE[key]
    res = run_bass_kernel_spmd(nc, in_maps, list(range(NC)))
    scol = meta["scol_of_local"]
    full = np.empty((N, F_OUT), np.float32)
    for c in range(NC):
        outc = np.asarray(res.results[c]["out"])     # [128, SPC]
        sc = scol[c, :SL]
        s, m = sc // SPC, sc % SPC
        full[c * SL:(c + 1) * SL] = outc[
            (16 * s[:, None] + np.arange(F_OUT)[None, :]), m[:, None]]
    return full

